# revision 54
# baseline (speedup 1.0000x reference)
"""Trainium2 Bass kernel for the DiscretisedDiffusion histogram-binning problem.

Math (reference):
    inp = cat([mu, t])                       # [2K+1], K=8192
    h   = leaky_relu(inp @ W1 + b1, 0.01)    # [2048]
    out = h @ W2 + b2                        # [2K]
    mu_eps, ln_sig = out[:K], out[K:]
    mu_x    = mu[:K]^p_mu * mu_eps^p_eps         (p_mu = g - 1/(1-g), p_eps = 1/(1-g))
    sigma_x = (1-g)^-0.5 * exp(0.5 ln_sig)
    edges e_j = 2(j-1)/(K-1); F(x) = clamp-masked 0.5(1+erf((x-mu_x)/(sigma_x sqrt2)))
    result[d, k] = F(e_{k+1}) - F(e_k)       # [K, K]

Key structure exploited:
  - kl[k] = kr[k-1], so one erf grid of 4097 edge columns serves both CDFs.
  - For k >= 4097 both CDFs clamp to 1 -> right half of the output is exactly 0
    (left unwritten; run_bass_kernel_spmd pre-zeros ExternalOutput buffers).
  - col 4096 uses a virtual right edge with F = 1.
  - The 0.5 factor of the CDF is exact in fp, so the device emits raw
    erf-differences and the host multiplies by 0.5 during the unshard gather.

Performance structure (the kernel is HBM-DMA bound; ~360 GB/s/core shared bus):
  - W1, W2, x, h in fp16 (halves weight traffic; rel err ~1e-4).
  - Output in bf16 (halves output traffic; rel err ~1e-3 vs 2e-2 budget).
  - W2 is laid out host-side in per-row-tile column blocks [128, 16*256] so
    each 128-row tile's (mu_eps, ln_sig) columns finalize right after its
    block lands -> the W2 stream overlaps the erf/sub/store grid phase.
  - Weight loads issue on the ACT HWDGE queue, output stores + collective
    bounces on the SP queue: two independent in-order rings, no
    head-of-line blocking between loads (always ready) and stores
    (ready only after their subtract).

Sharding (8 cores): output rows d split 1024/core. W1 sharded over its
contraction dim (2048 rows/core; the t-row is folded in via a zero-padded
per-core xl scalar); partial h AllReduce-summed (8 KiB). W2/b2 sharded over
their output dim. Per-core HBM: ~8.4 MiB W1 + 8.4 MiB W2 + 8.4 MiB output.
"""

import sys

if "/opt/trn_rl_repo" not in sys.path:
    sys.path.insert(0, "/opt/trn_rl_repo")

import numpy as np

K_BINS = 8192
D = 2 * K_BINS          # 16384
HIDDEN = 2048
N_CORES = 8
RPC = K_BINS // N_CORES  # 1024 output rows per core
KPC = D // N_CORES       # 2048 W1 contraction rows per core
KT1 = KPC // 128         # 16 W1 k-tiles
KT2 = HIDDEN // 128      # 16 matvec2 k-tiles
NB = RPC // 128          # 8 row-tiles (= W2 column blocks) per core
NE = K_BINS // 2 + 1     # 4097 real edge columns (j = 0..4096)
H0 = NE // 2 + 1         # 2049 left-half res columns
H1 = NE - H0             # 2048 right-half res columns
DS = 256                 # right-half sub: DVE takes [0,DS), Pool [DS,H1)
SQRT2 = 1.4142135623730951
TMIN = 1e-10
LEAKY = 0.01

_prog_cache = {}


def _build_program(p_mu, p_eps, ln_c, use_nn, sqrt_mu_path, square_eps,
                   single_core=False):
    import concourse.bacc as bacc
    import concourse.tile as tile
    import concourse.mybir as mybir

    f32 = mybir.dt.float32
    f16 = mybir.dt.float16
    bf16 = mybir.dt.bfloat16
    AF = mybir.ActivationFunctionType
    OP = mybir.AluOpType

    nc = bacc.Bacc("TRN2", target_bir_lowering=False, debug=False,
                   num_devices=1 if single_core else N_CORES)

    # small per-core f32 inputs packed into one DMA:
    # cols [0:8) muT | [8:24) b1T | [24:40) b2T | [40:56) w1lT | [56] xl
    NMISC = NB + KT2 + KT2 + KT2 + 1
    misc_d = nc.dram_tensor("misc", [128, NMISC], f32, kind="ExternalInput")
    # fp16 x (mu slice for matvec1), partition-major [128, 16]
    misc16_d = nc.dram_tensor("misc16", [128, KT1], f16, kind="ExternalInput")
    w1_d = nc.dram_tensor("w1", [KT1, 128, HIDDEN], f16, kind="ExternalInput")
    # W2 column blocks: block r holds, partition-major over k,
    # cols q*256+[0:128) = mu_eps cols of row-tile r (k-tile q)
    # cols q*256+[128:256) = ln_sig cols of row-tile r
    w2_d = nc.dram_tensor("w2", [NB, 128, KT2 * 256], f16,
                          kind="ExternalInput")
    out_d = nc.dram_tensor("out", [RPC, K_BINS], bf16, kind="ExternalOutput")

    with tile.TileContext(nc) as tc:
        with (
            tc.tile_pool(name="const", bufs=1) as constp,
            tc.tile_pool(name="w1p", bufs=1) as w1p,
            tc.tile_pool(name="w2p", bufs=1) as w2p,
            tc.tile_pool(name="grid", bufs=1) as gp,
            tc.tile_pool(name="small", bufs=1) as sp,
            tc.tile_pool(name="psmv", bufs=1, space="PSUM") as psmv,
            tc.tile_pool(name="ps2p", bufs=2, space="PSUM") as ps2p,
            tc.tile_pool(name="dram", bufs=1, space="DRAM") as dramp,
        ):
            misc = constp.tile([128, NMISC], f32)
            nc.sync.dma_start(misc[:], misc_d[:])
            misc16 = constp.tile([128, KT1], f16)
            nc.sync.dma_start(misc16[:], misc16_d[:])
            mupow = misc[:, 0:8]     # mu^p_mu, computed host-side
            b1_sb = misc[:, 8:24]
            b2_sb = misc[:, 24:40]
            w1lT = misc[:, 40:56]
            xlb = misc[:, 56:57]
            xT = misc16[:, 0:16]

            # --- W1 stream (ACT HWDGE; nothing blocks these) ---
            w1ts = []
            if use_nn:
                for q in range(KT1):
                    wt = w1p.tile([128, HIDDEN], f16, tag=f"w1t{q}",
                                  name=f"w1t{q}")
                    nc.scalar.dma_start(wt[:], w1_d[q])
                    w1ts.append(wt)
            # --- W2 block prefetch helper (also ACT HWDGE) ---
            w2ts = {}

            def fetch_w2(r, engine, gate=None):
                t = w2p.tile([128, KT2 * 256], f16, tag=f"w2s{r % 4}",
                             name=f"w2b{r}")
                if gate is not None:
                    # artificial WAW dependency: the tile scheduler hoists
                    # dep-free DMA dispatches, and an ungated W2 block
                    # transfer would jump into the W1 stream on the shared
                    # DMA engines, delaying matvec1 (and so everything)
                    nc.vector.tensor_copy(t[0:1, 0:1], gate)
                engine.dma_start(t[:], w2_d[r])
                w2ts[r] = t

            if use_nn:
                # Only block 0 is dispatched upfront, on the ACT HWDGE
                # behind the W1 tiles. Blocks 1..7 issue on the SP queue
                # AFTER the h AllReduce bounce, so the tiny bounce DMAs
                # don't queue behind 12us of prefetched W2 on the shared
                # DMA engines (the bus serves transfers in ready order).
                fetch_w2(0, nc.scalar)

            # --- edge values generated on device: e_j = (2j - 2)/(K-1) ---
            ej_i32 = constp.tile([128, NE], mybir.dt.int32)
            nc.gpsimd.iota(ej_i32[:], [[1, NE]], base=0, channel_multiplier=0)
            edges_sb = constp.tile([128, NE], f32)
            nc.vector.tensor_scalar(
                edges_sb[:], ej_i32[:], 2.0 / (K_BINS - 1), -2.0 / (K_BINS - 1),
                op0=OP.mult, op1=OP.add)

            a_t = sp.tile([128, NB], f32)
            cb_t = sp.tile([128, NB], f32)
            # dummy activation pulls the one ACT table load (Sigmoid, Erf
            # and Copy share the sigmoid_and_others set) off the critical
            # path; mu^p_mu comes precomputed from the host so no
            # Sqrt/Ln/Exp set is ever touched
            tdum = sp.tile([128, 1], f32, name="tdum")
            nc.scalar.activation(tdum[:], b1_sb[:, 0:1], AF.Sigmoid)
            nc.scalar.activation(tdum[:], b1_sb[:, 0:1], AF.Erf)

            res0s = {}
            ems = {}

            def emit_grid_left(r):
                rows = slice(r * 128, (r + 1) * 128)
                E0 = gp.tile([128, H0 + 1], f32, tag="E0", bufs=2,
                             name=f"E0_{r}")
                nc.scalar.activation(E0[:], edges_sb[:, 0:H0 + 1],
                                     AF.Erf, scale=a_t[:, r:r + 1],
                                     bias=cb_t[:, r:r + 1])
                res0 = gp.tile([128, H0], bf16, tag="res0", bufs=2,
                               name=f"res0_{r}")
                nc.vector.tensor_sub(res0[:], E0[:, 1:H0 + 1], E0[:, 0:H0])
                nc.sync.dma_start(out_d[rows, 0:H0], res0[:])
                res0s[r] = res0

            def emit_grid_right(r):
                rows = slice(r * 128, (r + 1) * 128)
                E1 = gp.tile([128, H1 + 1], f32, tag="E1", bufs=2,
                             name=f"E1_{r}")
                nc.scalar.activation(E1[:, 0:H1], edges_sb[:, H0:NE],
                                     AF.Erf, scale=a_t[:, r:r + 1],
                                     bias=cb_t[:, r:r + 1])
                nc.gpsimd.memset(E1[:, H1:H1 + 1], 1.0)
                base = 0
                res1 = gp.tile([128, H1], bf16, tag="res1", bufs=2,
                               name=f"res1_{r}")
                # right-half subtract split DVE/Pool to balance the
                # per-block engine load (DVE also carries the left half);
                # the final blocks rebalance toward DVE so the program's
                # last store isn't tail-gated by the slower Pool subtract
                ds = 1152 if r >= NB - 2 else DS
                nc.vector.tensor_sub(res1[:, 0:ds],
                                     E1[:, base + 1:base + ds + 1],
                                     E1[:, base:base + ds])
                nc.gpsimd.tensor_sub(res1[:, ds:H1],
                                     E1[:, base + ds + 1:base + H1 + 1],
                                     E1[:, base + ds:base + H1])
                nc.sync.dma_start(out_d[rows, H0:NE], res1[:])

            if use_nn:
                # lncb[:, r] = ln_c - 0.5*b2_ln[r]: folds the ln_sig bias
                # into the sigmoid's per-partition bias operand
                lncb = sp.tile([128, NB], f32, name="lncb")
                nc.vector.tensor_scalar(lncb[:], b2_sb[:, NB:2 * NB],
                                        -0.5, ln_c, op0=OP.mult, op1=OP.add)
                ident11 = sp.tile([128, 1], f32, name="ident11")
                nc.vector.memset(ident11[:], 1.0)

                # --- matvec1: partial h over this core's W1 rows ---
                # Free-major form: the x column is the (tiny) stationary
                # operand and the W tile streams through the moving port,
                # so each fp16 matmul costs one cheap Ldweights + N moving
                # cycles instead of a [128,128] stationary load per k-tile
                # (tile_legalize pairs every non-f32 matmul with an
                # InstLdweights, and its PE-sequencer slot is the real
                # cost at high matmul counts). PSUM lands free-major
                # [1, 2048]; PE transposes restore partition-major.
                # q-outer / c-inner: the four 512-col chunks accumulate in
                # four DIFFERENT 2KB psum zero regions, so their groups may
                # legally interleave, and the PE consumes each W1 tile as
                # it lands instead of re-walking all tiles per chunk.
                ps1 = psmv.tile([1, HIDDEN], f32, tag="ps1", name="ps1")
                for q in range(KT1):
                    for c in range(HIDDEN // 512):
                        nc.tensor.matmul(
                            ps1[0:1, c * 512:(c + 1) * 512],
                            xT[:, q:q + 1],
                            w1ts[q][:, c * 512:(c + 1) * 512],
                            start=(q == 0), stop=(q == KT1 - 1))
                # psum -> SBUF (halves on ACT and DVE in parallel) ->
                # transposed psum
                hp1 = sp.tile([1, HIDDEN], f32, name="hp1")
                hpT2 = psmv.tile([128, KT2], f32, tag="hpT2", name="hpT2")
                nc.scalar.activation(hp1[0:1, 0:1024], ps1[0:1, 0:1024],
                                     AF.Copy)
                nc.vector.tensor_copy(hp1[0:1, 1024:2048],
                                      ps1[0:1, 1024:2048])
                for j in range(KT2):
                    nc.tensor.transpose(hpT2[:, j:j + 1],
                                        hp1[0:1, j * 128:(j + 1) * 128],
                                        ident11[0:1, 0:1])
                # t-row contribution: xl * W1[D, :] (xl nonzero on one core
                # only, so it can be added before the AllReduce)
                tcon = sp.tile([128, KT2], f32, name="tcon")
                nc.vector.tensor_scalar_mul(tcon[:], w1lT, xlb)
                hpT = sp.tile([128, KT2], f32, name="hpT")
                nc.vector.tensor_add(hpT[:], tcon[:], hpT2[:])

                hp_dram = dramp.tile([128, KT2], f32)
                hs_dram = dramp.tile([128, KT2], f32)
                nc.sync.dma_start(hp_dram[:], hpT[:])
                if single_core:
                    # timing stand-in for the AllReduce (TimelineSim has no
                    # collectives); same DRAM bounce pattern
                    nc.sync.dma_start(hs_dram[:], hp_dram[:])
                else:
                    nc.gpsimd.collective_compute(
                        "AllReduce", OP.add,
                        replica_groups=[list(range(N_CORES))],
                        ins=[hp_dram.opt()], outs=[hs_dram.opt()])
                hT = sp.tile([128, KT2], f32)
                nc.sync.dma_start(hT[:], hs_dram[:])
                # h = leaky_relu(h + b1) = max(0.01*(h+b1), h+b1), in place
                nc.vector.tensor_add(hT[:], hT[:], b1_sb[:])
                nc.vector.scalar_tensor_tensor(
                    hT[:], hT[:], LEAKY, hT[:], op0=OP.mult, op1=OP.max)
                # blocks 1..3 gated on hT (the transfers follow the hs
                # bounce read immediately, without waiting the DVE chain)
                for r in range(1, 4):
                    fetch_w2(r, nc.sync, gate=hT[0:1, 0:1])
                hT16 = sp.tile([128, KT2], f16, name="hT16")
                nc.vector.tensor_copy(hT16[:], hT[:])

                # --- per row-tile: matvec2 block -> a/cb -> erf grid ---
                # Free-major again: out [1, 256] accumulates in a bank-
                # aligned 512-col slice of ps1 (matvec1's banks, free by
                # now), then an ACT copy + 2 PE transposes restore
                # [128, 2] partition-major.
                def chain(r):
                    """matvec2 + a/cb computation for row-tile r."""
                    base2 = (r % 4) * 512
                    o2 = sp.tile([1, 256], f32, tag="o2", bufs=2,
                                 name=f"o2_{r}")
                    ps2 = ps2p.tile([128, 2], f32, tag="ps2",
                                    name=f"ps2_{r}")
                    if r == 0:
                        # block 0 is latency-critical: compute the ln_sig
                        # half (j=1) first so the sigmoid->a->cb chain
                        # starts while the mu_eps half still multiplies
                        for j in (1, 0):
                            ps2h = ps1[0:1, base2 + j * 128:
                                       base2 + (j + 1) * 128]
                            for q in range(KT2):
                                nc.tensor.matmul(
                                    ps2h,
                                    hT16[:, q:q + 1],
                                    w2ts[r][:, q * 256 + j * 128:
                                            q * 256 + (j + 1) * 128],
                                    start=(q == 0), stop=(q == KT2 - 1))
                            nc.vector.tensor_copy(
                                o2[0:1, j * 128:(j + 1) * 128], ps2h)
                            nc.tensor.transpose(
                                ps2[:, j:j + 1],
                                o2[0:1, j * 128:(j + 1) * 128],
                                ident11[0:1, 0:1])
                    else:
                        ps2f = ps1[0:1, base2:base2 + 256]
                        for q in range(KT2):
                            nc.tensor.matmul(
                                ps2f,
                                hT16[:, q:q + 1],
                                w2ts[r][:, q * 256:(q + 1) * 256],
                                start=(q == 0), stop=(q == KT2 - 1))
                        nc.vector.tensor_copy(o2[:], ps2f)
                        for j in range(2):
                            nc.tensor.transpose(ps2[:, j:j + 1],
                                                o2[0:1, j * 128:(j + 1) * 128],
                                                ident11[0:1, 0:1])
                    eps = sp.tile([128, 1], f32, tag="eps", bufs=2,
                                  name=f"eps_{r}")
                    nc.vector.tensor_add(eps[:], ps2[:, 0:1],
                                         b2_sb[:, r:r + 1])
                    # a = 1/(sigma_x sqrt2) = exp(-0.5 lns + ln_c), via the
                    # sigmoid table: e^y = s/(1-s), s = sigma(y); the b2
                    # part of lns rides in via the lncb bias
                    s0 = sp.tile([128, 1], f32, tag="s0", bufs=2,
                                 name=f"s0_{r}")
                    nc.scalar.activation(s0[:], ps2[:, 1:2], AF.Sigmoid,
                                         scale=-0.5, bias=lncb[:, r:r + 1])
                    om = sp.tile([128, 1], f32, tag="om", bufs=2,
                                 name=f"om_{r}")
                    nc.vector.tensor_scalar(om[:], s0[:], -1.0, 1.0,
                                            op0=OP.mult, op1=OP.add)
                    nc.vector.reciprocal(om[:], om[:])
                    nc.vector.tensor_mul(a_t[:, r:r + 1], s0[:], om[:])
                    # mu_x = mu^p_mu * eps^p_eps ; cb = -mu_x * a
                    # (small ops on the idle Pool engine, off the DVE path)
                    epspow = sp.tile([128, 1], f32, tag="epspow", bufs=2,
                                     name=f"epspow_{r}")
                    if square_eps:
                        nc.vector.tensor_mul(epspow[:], eps[:], eps[:])
                    else:
                        lneps = sp.tile([128, 1], f32, tag="lneps", bufs=2,
                                        name=f"lneps_{r}")
                        nc.scalar.activation(lneps[:], eps[:], AF.Ln)
                        nc.scalar.activation(epspow[:], lneps[:], AF.Exp,
                                             scale=p_eps)
                    # mupow holds -mu^p_mu (negated host-side), so
                    # cb = -mu_x*a needs only two Pool tensor_muls
                    # (scalar_tensor_tensor is not a valid Pool opcode)
                    mux = sp.tile([128, 1], f32, tag="mux", bufs=2,
                                  name=f"mux_{r}")
                    nc.vector.tensor_mul(mux[:], mupow[:, r:r + 1], epspow[:])
                    nc.vector.tensor_mul(cb_t[:, r:r + 1], mux[:],
                                         a_t[:, r:r + 1])

                # Software pipeline: block r+1's matvec/a/cb work is
                # emitted BETWEEN block r's two grid halves, so block r's
                # first subtract starts immediately while block r+1's
                # small DVE/ACT ops still land one block ahead of its
                # erfs in the in-order engine queues.
                chain(0)
                for r in range(NB):
                    emit_grid_left(r)
                    if r + 4 < NB:
                        # gated on block r's just-written left result so
                        # the W2 block transfers INTERLEAVE with the out
                        # stores on the bus (instead of all-W2-first,
                        # which starves the res-tile slot recycling)
                        fetch_w2(r + 4, nc.sync, gate=res0s[r][0:1, 0:1])
                    if r + 1 < NB:
                        chain(r + 1)
                    emit_grid_right(r)
            else:
                # t < tmin: mu_x = 0, sigma_x = 1 -> erf(x/sqrt2)
                nc.vector.memset(a_t[:], 1.0 / SQRT2)
                nc.vector.memset(cb_t[:], 0.0)
                for r in range(NB):
                    emit_grid_row(r)

    nc.compile()
    return nc


def _prep_inputs(mu, t, W1, b1, W2, b2, tval, use_nn, p_mu):
    mu = np.ascontiguousarray(mu, np.float32)
    b1 = np.ascontiguousarray(b1, np.float32)
    b2 = np.ascontiguousarray(b2, np.float32)

    W1_16 = W1[:D].astype(np.float16)         # [D, HIDDEN]
    W2_16 = W2.astype(np.float16)             # [HIDDEN, 2K]
    w1lT = np.ascontiguousarray(
        np.asarray(W1[D], np.float32).reshape(KT2, 128).T)
    b1T = np.ascontiguousarray(b1.reshape(KT2, 128).T)
    in_maps = []
    for c in range(N_CORES):
        xtT = mu[c * KPC:(c + 1) * KPC].reshape(KT1, 128).T.astype(np.float16)
        xlv = tval if c == N_CORES - 1 else 0.0

        w1blk = np.ascontiguousarray(
            W1_16[c * KPC:(c + 1) * KPC].reshape(KT1, 128, HIDDEN))

        # W2 column blocks: blk[r][p, q*256 + j*128 + i] =
        #   W2[q*128+p, (j ? K : 0) + c*RPC + r*128 + i]
        muc = W2_16[:, c * RPC:(c + 1) * RPC].reshape(KT2, 128, NB, 128)
        lnc = W2_16[:, K_BINS + c * RPC:K_BINS + (c + 1) * RPC].reshape(
            KT2, 128, NB, 128)
        # -> [r, p, q, j, i]
        w2blk = np.ascontiguousarray(
            np.stack([muc, lnc], axis=2).transpose(3, 1, 0, 2, 4).reshape(
                NB, 128, KT2 * 256))

        b2blk = np.concatenate(
            [b2[c * RPC:(c + 1) * RPC],
             b2[K_BINS + c * RPC:K_BINS + (c + 1) * RPC]])

        # negated so the device's cb = -mu_x*a is two plain multiplies
        mupowT = (-(mu[c * RPC:(c + 1) * RPC].astype(np.float64) ** p_mu)
                  ).astype(np.float32).reshape(NB, 128).T
        misc = np.concatenate([
            mupowT, b1T, b2blk.reshape(2 * NB, 128).T, w1lT,
            np.full((128, 1), xlv, np.float32)], axis=1)

        in_maps.append({
            "misc": np.ascontiguousarray(misc, np.float32),
            "misc16": np.ascontiguousarray(xtT),
            "w1": w1blk,
            "w2": w2blk,
        })
    return in_maps


def kernel(mu, t, gamma, W1, b1, W2, b2, K=None, **_unused):
    from concourse.bass_utils import run_bass_kernel_spmd

    assert K is None or int(K) == K_BINS

    g = float(np.asarray(gamma, np.float64).reshape(-1)[0])
    tval = float(np.asarray(t, np.float64).reshape(-1)[0])
    p_mu = g - 1.0 / (1.0 - g)
    p_eps = 1.0 / (1.0 - g)
    use_nn = bool(tval >= TMIN)
    ln_c = 0.5 * np.log1p(-g) - 0.5 * np.log(2.0)
    sqrt_mu_path = abs(p_mu + 1.5) < 1e-12
    square_eps = abs(p_eps - 2.0) < 1e-12

    key = (round(p_mu, 12), round(p_eps, 12), round(ln_c, 12), use_nn)
    if key not in _prog_cache:
        _prog_cache[key] = _build_program(
            p_mu, p_eps, float(ln_c), use_nn, sqrt_mu_path, square_eps)
    nc = _prog_cache[key]

    in_maps = _prep_inputs(mu, t, W1, b1, W2, b2, tval, use_nn, p_mu)
    res = run_bass_kernel_spmd(nc, in_maps, list(range(N_CORES)))
    # device emits raw erf differences in bf16; the CDF's 0.5 factor is
    # exact, so apply it on the host during the f32 gather
    out = np.concatenate(
        [np.asarray(res.results[c]["out"]).astype(np.float32)
         for c in range(N_CORES)], axis=0)
    out *= 0.5
    return out
